# revision 2
# baseline (speedup 1.0000x reference)
"""Cayley orthogonal transform kernel for Trainium2 (8 NeuronCores).

Math: per head h, y = (I - S) ((1+eps) I + S)^{-1} x applied along D=128,
where S = S_raw - S_raw^T is skew-symmetric.

Strategy:
  * Host: skew-symmetrize S_raw, and lay x out as xT[h, d, token] (token-major
    per head) so the device only ever runs plain matmuls - no on-device
    transposes.  Heads are sharded 2-per-core across the 8 cores (tensor
    parallel, embarrassingly parallel per the problem structure).
  * Device (per core): build W^T = ((1+eps)I - S)^{-1} (I + S) per head with a
    Newton-Schulz iteration (pure 128x128 matmuls, converges far past bf16
    accuracy in 5 iterations since ||S||_2 ~ 1.6), then stream the
    (128 x 16384) token panel through the PE array in 512-column bf16 matmuls:
        yT[h] = W @ xT[h]
    PSUM results are evacuated to bf16 SBUF tiles alternating Vector/Scalar
    engines and DMA'd back to DRAM.  Everything is bf16 over the wire (x in,
    y out, W in the PE): the accuracy budget is rel_l2 < 2e-2 and bf16
    end-to-end lands at ~3e-3, so the kernel runs at the 2-byte HBM roofline
    (~17 MB of DRAM traffic per core) with all compute hidden under the DMA.
  * Host: widen y to fp32 and inverse layout transform back to (B, H, N, D).
"""

import os
import sys

import numpy as np

B, H, N, D = 4, 16, 4096, 128
N_CORES = 8
HPC = H // N_CORES          # heads per core
T = B * N                   # tokens per head
CHUNK = 4096                # columns per DMA tile (1 MiB bf16)
MM = 512                    # columns per matmul (one PSUM bank)
NS_ITERS = 5                # Newton-Schulz iterations
NS_C = 0.42                 # NS initial scale: X0 = c * G^T  (safe for ||S||<~1.9)
EPS = 1e-5

_CACHE = {}


def _ensure_path():
    for p in ("/opt/trn_rl_repo", "/root/.axon_site/_ro/trn_rl_repo"):
        if os.path.isdir(p) and p not in sys.path:
            sys.path.insert(0, p)
    _install_ntff_hook()


def _install_ntff_hook():
    """The agent image's ``antenv`` lacks ``axon_hooks``, which makes
    ``run_bass_kernel_spmd(trace=True)`` crash instead of degrading.  Provide
    the module and register the ctypes NTFF hook the boot shim would have."""
    if "antenv.axon_hooks" in sys.modules:
        return
    try:
        import types

        import antenv

        if hasattr(antenv, "axon_hooks"):
            return
        mod = types.ModuleType("antenv.axon_hooks")
        state = {"hook": None}
        mod.set_axon_ntff_profile_hook = lambda h: state.__setitem__("hook", h)
        mod.get_axon_ntff_profile_hook = lambda: state["hook"]
        sys.modules["antenv.axon_hooks"] = mod
        antenv.axon_hooks = mod
        try:
            from trn_agent_boot.trn_boot import _ntff_profile_via_ctypes

            so_path = "/opt/axon/libaxon_pjrt.so"
            if os.path.exists(so_path):
                mod.set_axon_ntff_profile_hook(_ntff_profile_via_ctypes(so_path))
        except Exception:
            pass  # hook stays None -> concourse logs + skips tracing
    except Exception:
        pass


def _build_nc():
    """Build the (single-program SPMD) Bass kernel for one core's shard."""
    _ensure_path()
    import concourse.tile as tile
    from concourse import bacc, mybir
    from concourse.masks import make_identity

    f32 = mybir.dt.float32
    bf16 = mybir.dt.bfloat16
    Alu = mybir.AluOpType

    nc = bacc.Bacc("TRN2", target_bir_lowering=False, debug=False)
    x_d = nc.dram_tensor("xhi", [HPC * D, T], bf16, kind="ExternalInput").ap()
    s_d = nc.dram_tensor("s", [HPC * D, D], f32, kind="ExternalInput").ap()
    yT_d = nc.dram_tensor("yT", [HPC * D, T], bf16, kind="ExternalOutput").ap()

    with tile.TileContext(nc) as tc:
        with (
            tc.tile_pool(name="const", bufs=1) as const_pool,
            tc.tile_pool(name="ns", bufs=2) as ns_pool,
            tc.tile_pool(name="wt", bufs=1) as wt_pool,
            tc.tile_pool(name="xin", bufs=6) as in_pool,
            tc.tile_pool(name="yout", bufs=3) as out_pool,
            tc.tile_pool(name="mmps", bufs=4, space="PSUM") as ps_big,
            tc.tile_pool(name="nsps", bufs=4, space="PSUM") as ps_ns,
        ):
            ident = const_pool.tile([D, D], f32, tag="ident")
            make_identity(nc, ident)
            twoE = const_pool.tile([D, D], f32, tag="twoE")
            nc.vector.tensor_scalar_mul(twoE, ident, 2.0)

            # --- Newton-Schulz per head: WT = Ginv @ (I + S), G = (1+eps)I - S
            # bass matmul computes lhsT.T @ rhs; note A := (1+eps)I + S = G^T.
            wts = []
            for h in range(HPC):
                s_sb = const_pool.tile([D, D], f32, tag=f"s{h}")
                nc.sync.dma_start(out=s_sb, in_=s_d[h * D:(h + 1) * D, :])
                a_mat = const_pool.tile([D, D], f32, tag=f"amat{h}")
                nc.vector.scalar_tensor_tensor(
                    out=a_mat, in0=ident, scalar=1.0 + EPS, in1=s_sb,
                    op0=Alu.mult, op1=Alu.add)
                ips = const_pool.tile([D, D], f32, tag=f"ips{h}")
                nc.vector.tensor_add(ips, ident, s_sb)
                g_mat = const_pool.tile([D, D], f32, tag=f"g{h}")
                nc.vector.scalar_tensor_tensor(
                    out=g_mat, in0=ident, scalar=1.0 + EPS, in1=s_sb,
                    op0=Alu.mult, op1=Alu.subtract)

                X = ns_pool.tile([D, D], f32, tag=f"x{h}")
                nc.vector.tensor_scalar_mul(X, a_mat, NS_C)    # X0 = c G^T
                XT = ns_pool.tile([D, D], f32, tag=f"xt{h}")
                nc.vector.tensor_scalar_mul(XT, g_mat, NS_C)   # X0^T = c G

                for k in range(NS_ITERS):
                    t_ps = ps_ns.tile([D, D], f32, tag="nsps")
                    nc.tensor.matmul(t_ps, lhsT=a_mat, rhs=X, start=True, stop=True)  # G X
                    t2 = ns_pool.tile([D, D], f32, tag=f"t2{h}")
                    nc.vector.tensor_sub(t2, twoE, t_ps)       # 2I - G X
                    if k < NS_ITERS - 1:
                        xn_ps = ps_ns.tile([D, D], f32, tag="nsps")
                        nc.tensor.matmul(xn_ps, lhsT=XT, rhs=t2, start=True, stop=True)  # X T2
                        Xn = ns_pool.tile([D, D], f32, tag=f"x{h}")
                        nc.scalar.copy(Xn, xn_ps)
                        X = Xn
                    xtn_ps = ps_ns.tile([D, D], f32, tag="nsps")
                    nc.tensor.matmul(xtn_ps, lhsT=t2, rhs=XT, start=True, stop=True)  # (X T2)^T
                    XTn = ns_pool.tile([D, D], f32, tag=f"xt{h}")
                    nc.scalar.copy(XTn, xtn_ps)
                    XT = XTn

                wt_ps = ps_ns.tile([D, D], f32, tag="nsps")
                nc.tensor.matmul(wt_ps, lhsT=XT, rhs=ips, start=True, stop=True)  # Ginv (I+S)
                whi = wt_pool.tile([D, D], bf16, tag=f"whi{h}")
                nc.vector.tensor_copy(whi, wt_ps)
                wts.append(whi)

            # --- streaming panel matmul: yT[h] = W @ xT[h], all-bf16 single term
            half = CHUNK // 2
            for h in range(HPC):
                whi = wts[h]
                r0 = h * D
                for ci in range(T // CHUNK):
                    c0 = ci * CHUNK
                    xh = in_pool.tile([D, CHUNK], bf16, tag="xh")
                    nc.sync.dma_start(out=xh, in_=x_d[r0:r0 + D, c0:c0 + CHUNK])
                    yout = out_pool.tile([D, CHUNK], bf16, tag="yout")
                    for j in range(CHUNK // MM):
                        sl = slice(j * MM, (j + 1) * MM)
                        ps = ps_big.tile([D, MM], f32, tag="mm")
                        nc.tensor.matmul(ps, lhsT=whi, rhs=xh[:, sl],
                                         start=True, stop=True)
                        if j % 2 == 0:
                            nc.vector.tensor_copy(yout[:, sl], ps)
                        else:
                            nc.scalar.copy(yout[:, sl], ps)
                    # two half-stores so the DMA overlaps the second half's evac
                    nc.scalar.dma_start(out=yT_d[r0:r0 + D, c0:c0 + half],
                                        in_=yout[:, 0:half])
                    nc.scalar.dma_start(out=yT_d[r0:r0 + D, c0 + half:c0 + CHUNK],
                                        in_=yout[:, half:CHUNK])
    nc.compile()
    return nc


def _get_nc():
    if "nc" not in _CACHE:
        _CACHE["nc"] = _build_nc()
    return _CACHE["nc"]


def _prep_inputs(x, S_raw):
    """Host-side shard + layout prep. Returns per-core input maps."""
    import ml_dtypes

    bf16 = ml_dtypes.bfloat16
    x = np.asarray(x, dtype=np.float32)
    S_raw = np.asarray(S_raw, dtype=np.float32)
    S = S_raw - S_raw.transpose(0, 2, 1)
    # (B,H,N,D) -> (H, D, B*N), token-major per head; single bf16 copy
    xT_full = np.ascontiguousarray(x.transpose(1, 3, 0, 2)).reshape(H * D, T)
    xhi = xT_full.astype(bf16)
    S_full = np.ascontiguousarray(S).reshape(H * D, D)
    in_maps = []
    for c in range(N_CORES):
        r = c * HPC * D
        in_maps.append({
            "xhi": xhi[r:r + HPC * D],
            "s": S_full[r:r + HPC * D],
        })
    return in_maps


def _postprocess(results):
    """Gather per-core yT shards back into (B, H, N, D) fp32."""
    yT_full = np.concatenate([r["yT"] for r in results], axis=0)  # (H*D, T) bf16
    y = yT_full.astype(np.float32).reshape(H, D, B, N).transpose(2, 0, 3, 1)
    return np.ascontiguousarray(y)


def _execute(in_maps, trace=False, **kwargs):
    _ensure_path()
    from concourse.bass_utils import run_bass_kernel_spmd

    nc = _get_nc()
    return run_bass_kernel_spmd(nc, in_maps, core_ids=list(range(N_CORES)),
                                trace=trace, **kwargs)


def kernel(x, S_raw):
    in_maps = _prep_inputs(x, S_raw)
    res = _execute(in_maps)
    return _postprocess(res.results)


# revision 3
# speedup vs baseline: 1.3504x; 1.3504x over previous
"""Cayley orthogonal transform kernel for Trainium2 (8 NeuronCores).

Math: per head h, y = (I - S) ((1+eps) I + S)^{-1} x applied along D=128,
where S = S_raw - S_raw^T is skew-symmetric.

Strategy:
  * Host: skew-symmetrize S_raw and precompute the per-head Cayley weight
    W^T = ((1+eps)I - S)^{-1} (I + S)  (parameter-only, O(H D^3) = 0.2% of
    total FLOPs; independent of x).  Lay x out as xT[h, d, token]
    (token-major per head) so the device only ever runs plain matmuls - no
    on-device transposes.  Heads are sharded 2-per-core across the 8 cores
    (tensor parallel, embarrassingly parallel per the problem structure).
  * Device (per core): pure streaming panel matmul yT[h] = W @ xT[h].
    All of x (8 MiB bf16) is loaded into SBUF up-front with 8 large 1 MiB
    DMAs that saturate the 16 DMA queues from t=0; the PE array streams
    512-column bf16 matmuls out of those resident tiles, PSUM is evacuated
    to bf16 SBUF alternating Vector/Scalar engines, and finished 1 MiB
    output tiles are DMA'd back.  Everything is bf16 over the wire (x in,
    y out, W in the PE): the accuracy budget is rel_l2 < 2e-2 and bf16
    end-to-end lands at ~3e-3, so the kernel runs at the 2-byte HBM
    roofline (~17 MB of DRAM traffic per core).
  * Host: widen y to fp32 and inverse layout transform back to (B, H, N, D).
"""

import os
import sys

import numpy as np

B, H, N, D = 4, 16, 4096, 128
N_CORES = 8
HPC = H // N_CORES          # heads per core
T = B * N                   # tokens per head
CHUNK = 4096                # columns per SBUF tile / DMA (1 MiB bf16)
MM = 512                    # columns per matmul (one PSUM bank)
NTILE = HPC * T // CHUNK    # resident x tiles per core (8)
EPS = 1e-5

_CACHE = {}


def _ensure_path():
    for p in ("/opt/trn_rl_repo", "/root/.axon_site/_ro/trn_rl_repo"):
        if os.path.isdir(p) and p not in sys.path:
            sys.path.insert(0, p)
    _install_ntff_hook()


def _install_ntff_hook():
    """The agent image's ``antenv`` lacks ``axon_hooks``, which makes
    ``run_bass_kernel_spmd(trace=True)`` crash instead of degrading.  Provide
    the module and register the ctypes NTFF hook the boot shim would have."""
    if "antenv.axon_hooks" in sys.modules:
        return
    try:
        import types

        import antenv

        if hasattr(antenv, "axon_hooks"):
            return
        mod = types.ModuleType("antenv.axon_hooks")
        state = {"hook": None}
        mod.set_axon_ntff_profile_hook = lambda h: state.__setitem__("hook", h)
        mod.get_axon_ntff_profile_hook = lambda: state["hook"]
        sys.modules["antenv.axon_hooks"] = mod
        antenv.axon_hooks = mod
        try:
            from trn_agent_boot.trn_boot import _ntff_profile_via_ctypes

            so_path = "/opt/axon/libaxon_pjrt.so"
            if os.path.exists(so_path):
                mod.set_axon_ntff_profile_hook(_ntff_profile_via_ctypes(so_path))
        except Exception:
            pass  # hook stays None -> concourse logs + skips tracing
    except Exception:
        pass


def _build_nc():
    """Build the (single-program SPMD) Bass kernel for one core's shard."""
    _ensure_path()
    import concourse.tile as tile
    from concourse import bacc, mybir

    bf16 = mybir.dt.bfloat16
    f32 = mybir.dt.float32

    nc = bacc.Bacc("TRN2", target_bir_lowering=False, debug=False)
    x_d = nc.dram_tensor("xh", [HPC * D, T], bf16, kind="ExternalInput").ap()
    wt_d = nc.dram_tensor("wt", [HPC * D, D], bf16, kind="ExternalInput").ap()
    yT_d = nc.dram_tensor("yT", [HPC * D, T], bf16, kind="ExternalOutput").ap()

    tiles_per_head = T // CHUNK

    with tile.TileContext(nc) as tc:
        with (
            tc.tile_pool(name="wt", bufs=1) as wt_pool,
            tc.tile_pool(name="xin", bufs=1) as in_pool,
            tc.tile_pool(name="yout", bufs=1) as out_pool,
            tc.tile_pool(name="mmps", bufs=8, space="PSUM") as ps_pool,
        ):
            # W first (tiny), then flood the queues with all of x.
            wts = []
            for h in range(HPC):
                w_sb = wt_pool.tile([D, D], bf16, tag=f"w{h}")
                nc.sync.dma_start(out=w_sb, in_=wt_d[h * D:(h + 1) * D, :])
                wts.append(w_sb)
            xts = []
            for i in range(NTILE):
                h, ci = divmod(i, tiles_per_head)
                xt = in_pool.tile([D, CHUNK], bf16, tag=f"x{i}")
                nc.sync.dma_start(
                    out=xt,
                    in_=x_d[h * D:(h + 1) * D, ci * CHUNK:(ci + 1) * CHUNK])
                xts.append(xt)

            for i in range(NTILE):
                h, ci = divmod(i, tiles_per_head)
                yt = out_pool.tile([D, CHUNK], bf16, tag=f"y{i}")
                for j in range(CHUNK // MM):
                    sl = slice(j * MM, (j + 1) * MM)
                    ps = ps_pool.tile([D, MM], f32, tag="mm")
                    nc.tensor.matmul(ps, lhsT=wts[h], rhs=xts[i][:, sl],
                                     start=True, stop=True)
                    if j % 2 == 0:
                        nc.vector.tensor_copy(yt[:, sl], ps)
                    else:
                        nc.scalar.copy(yt[:, sl], ps)
                nc.scalar.dma_start(
                    out=yT_d[h * D:(h + 1) * D, ci * CHUNK:(ci + 1) * CHUNK],
                    in_=yt)
    nc.compile()
    return nc


def _get_nc():
    if "nc" not in _CACHE:
        _CACHE["nc"] = _build_nc()
    return _CACHE["nc"]


def _prep_inputs(x, S_raw):
    """Host-side shard + layout prep. Returns per-core input maps."""
    import ml_dtypes

    bf16 = ml_dtypes.bfloat16
    x = np.asarray(x, dtype=np.float32)
    S_raw = np.asarray(S_raw, dtype=np.float32)
    S = S_raw - S_raw.transpose(0, 2, 1)
    I = np.eye(D, dtype=np.float32)
    # lhsT for out = lhsT.T @ x  with lhsT.T = W = (I-S) A^{-1}:
    # lhsT = W^T = A^{-T} (I-S)^T = ((1+eps)I - S)^{-1} (I + S)
    WT = np.linalg.solve((1.0 + EPS) * I[None] - S, I[None] + S)  # (H, D, D)
    WT_full = WT.reshape(H * D, D).astype(bf16)
    # (B,H,N,D) -> (H, D, B*N), token-major per head; single bf16 copy
    xT_full = np.ascontiguousarray(x.transpose(1, 3, 0, 2)).reshape(H * D, T)
    xh = xT_full.astype(bf16)
    in_maps = []
    for c in range(N_CORES):
        r = c * HPC * D
        in_maps.append({
            "xh": xh[r:r + HPC * D],
            "wt": WT_full[r:r + HPC * D],
        })
    return in_maps


def _postprocess(results):
    """Gather per-core yT shards back into (B, H, N, D) fp32."""
    yT_full = np.concatenate([r["yT"] for r in results], axis=0)  # (H*D, T) bf16
    y = yT_full.astype(np.float32).reshape(H, D, B, N).transpose(2, 0, 3, 1)
    return np.ascontiguousarray(y)


def _execute(in_maps, trace=False, **kwargs):
    _ensure_path()
    from concourse.bass_utils import run_bass_kernel_spmd

    nc = _get_nc()
    return run_bass_kernel_spmd(nc, in_maps, core_ids=list(range(N_CORES)),
                                trace=trace, **kwargs)


def kernel(x, S_raw):
    in_maps = _prep_inputs(x, S_raw)
    res = _execute(in_maps)
    return _postprocess(res.results)


# revision 7
# speedup vs baseline: 1.4044x; 1.0400x over previous
"""Cayley orthogonal transform kernel for Trainium2 (8 NeuronCores).

Math: per head h, y = (I - S) ((1+eps) I + S)^{-1} x applied along D=128,
where S = S_raw - S_raw^T is skew-symmetric.

Strategy:
  * Host: skew-symmetrize S_raw and precompute the per-head Cayley weight
    W^T = ((1+eps)I - S)^{-1} (I + S)  (parameter-only, O(H D^3) = 0.2% of
    total FLOPs; independent of x).  Lay x out as xT[h, d, token]
    (token-major per head) so the device only ever runs plain matmuls - no
    on-device transposes.  Heads are sharded 2-per-core across the 8 cores
    (tensor parallel, embarrassingly parallel per the problem structure).
  * Device (per core): pure streaming panel matmul yT[h] = W @ xT[h].
    All of x (8 MiB bf16) is loaded into SBUF up-front with 8 large 1 MiB
    DMAs that saturate the 16 DMA queues from t=0; the PE array streams
    512-column bf16 matmuls out of those resident tiles, PSUM is evacuated
    to bf16 SBUF alternating Vector/Scalar engines, and finished 1 MiB
    output tiles are DMA'd back.  Everything is bf16 over the wire (x in,
    y out, W in the PE): the accuracy budget is rel_l2 < 2e-2 and bf16
    end-to-end lands at ~3e-3, so the kernel runs at the 2-byte HBM
    roofline (~17 MB of DRAM traffic per core).
  * Host: widen y to fp32 and inverse layout transform back to (B, H, N, D).
"""

import os
import sys

import numpy as np

B, H, N, D = 4, 16, 4096, 128
N_CORES = 8
HPC = H // N_CORES          # heads per core
T = B * N                   # tokens per head
MM = 512                    # columns per matmul (one PSUM bank)
# Graded x tile sizes per head: small first tile so the PE can start early,
# large tiles after to amortize DMA trigger/semaphore overhead.
XTILES = {0: (2048, 6144, 8192), 1: (8192, 8192)}
OUT_CHUNK = 4096            # columns per output store (1 MiB fp16)
EPS = 1e-5

_CACHE = {}


def _ensure_path():
    for p in ("/opt/trn_rl_repo", "/root/.axon_site/_ro/trn_rl_repo"):
        if os.path.isdir(p) and p not in sys.path:
            sys.path.insert(0, p)
    _install_ntff_hook()


def _install_ntff_hook():
    """The agent image's ``antenv`` lacks ``axon_hooks``, which makes
    ``run_bass_kernel_spmd(trace=True)`` crash instead of degrading.  Provide
    the module and register the ctypes NTFF hook the boot shim would have."""
    if "antenv.axon_hooks" in sys.modules:
        return
    try:
        import types

        import antenv

        if hasattr(antenv, "axon_hooks"):
            return
        mod = types.ModuleType("antenv.axon_hooks")
        state = {"hook": None}
        mod.set_axon_ntff_profile_hook = lambda h: state.__setitem__("hook", h)
        mod.get_axon_ntff_profile_hook = lambda: state["hook"]
        sys.modules["antenv.axon_hooks"] = mod
        antenv.axon_hooks = mod
        try:
            from trn_agent_boot.trn_boot import _ntff_profile_via_ctypes

            so_path = "/opt/axon/libaxon_pjrt.so"
            if os.path.exists(so_path):
                mod.set_axon_ntff_profile_hook(_ntff_profile_via_ctypes(so_path))
        except Exception:
            pass  # hook stays None -> concourse logs + skips tracing
    except Exception:
        pass


def _build_nc():
    """Build the (single-program SPMD) Bass kernel for one core's shard."""
    _ensure_path()
    import concourse.tile as tile
    from concourse import bacc, mybir

    f16 = mybir.dt.float16
    f32 = mybir.dt.float32

    nc = bacc.Bacc("TRN2", target_bir_lowering=False, debug=False)
    x_d = nc.dram_tensor("xh", [HPC * D, T], f16, kind="ExternalInput").ap()
    wt_d = nc.dram_tensor("wt", [D, HPC * D], f16, kind="ExternalInput").ap()
    yT_d = nc.dram_tensor("yT", [HPC * D, T], f16, kind="ExternalOutput").ap()

    with tile.TileContext(nc) as tc:
        with (
            tc.tile_pool(name="wt", bufs=1) as wt_pool,
            tc.tile_pool(name="xin", bufs=1) as in_pool,
            tc.tile_pool(name="yout", bufs=1) as out_pool,
            tc.tile_pool(name="mmps", bufs=8, space="PSUM") as ps_pool,
        ):
            # First x tile + the (tiny) W tile first, then the big x tiles.
            x0 = in_pool.tile([D, XTILES[0][0]], f16, tag="x0_0")
            nc.sync.dma_start(out=x0, in_=x_d[0:D, 0:XTILES[0][0]])
            w_sb = wt_pool.tile([D, HPC * D], f16, tag="w")
            nc.sync.dma_start(out=w_sb, in_=wt_d)
            xts = {0: [(0, x0)]}
            for h in range(HPC):
                c0 = XTILES[0][0] if h == 0 else 0
                for sz in XTILES[h][1 if h == 0 else 0:]:
                    xt = in_pool.tile([D, sz], f16, tag=f"x{h}_{c0}")
                    nc.sync.dma_start(
                        out=xt, in_=x_d[h * D:(h + 1) * D, c0:c0 + sz])
                    xts.setdefault(h, []).append((c0, xt))
                    c0 += sz

            for h in range(HPC):
                lhsT = w_sb[:, h * D:(h + 1) * D]
                # yout tiles per OUT_CHUNK columns; evac PSUM per MM columns
                yts = {}
                for c0, xt in xts[h]:
                    sz = xt.shape[-1]
                    for j in range(sz // MM):
                        col = c0 + j * MM          # absolute column in head
                        oc, oj = divmod(col, OUT_CHUNK)
                        if oj == 0:
                            yts[oc] = out_pool.tile([D, OUT_CHUNK], f16,
                                                    name=f"y{h}_{oc}",
                                                    tag=f"y{h}_{oc}")
                        ps = ps_pool.tile([D, MM], f32, tag="mm")
                        nc.tensor.matmul(ps, lhsT=lhsT,
                                         rhs=xt[:, j * MM:(j + 1) * MM],
                                         start=True, stop=True)
                        dst = yts[oc][:, oj:oj + MM]
                        if (col // MM) % 2 == 0:
                            nc.vector.tensor_copy(dst, ps)
                        else:
                            nc.scalar.copy(dst, ps)
                        if oj + MM == OUT_CHUNK:
                            nc.scalar.dma_start(
                                out=yT_d[h * D:(h + 1) * D,
                                         oc * OUT_CHUNK:(oc + 1) * OUT_CHUNK],
                                in_=yts[oc])
    nc.compile()
    return nc


def _get_nc():
    if "nc" not in _CACHE:
        _CACHE["nc"] = _build_nc()
    return _CACHE["nc"]


def _prep_inputs(x, S_raw):
    """Host-side shard + layout prep. Returns per-core input maps."""
    x = np.asarray(x, dtype=np.float32)
    S_raw = np.asarray(S_raw, dtype=np.float32)
    S = S_raw - S_raw.transpose(0, 2, 1)
    I = np.eye(D, dtype=np.float32)
    # lhsT for out = lhsT.T @ x  with lhsT.T = W = (I-S) A^{-1}:
    # lhsT = W^T = A^{-T} (I-S)^T = ((1+eps)I - S)^{-1} (I + S)
    WT = np.linalg.solve((1.0 + EPS) * I[None] - S, I[None] + S)  # (H, D, D)
    WT16 = WT.astype(np.float16)
    # (B,H,N,D) -> (H, D, B*N), token-major per head; single fp16 copy
    xT_full = np.ascontiguousarray(x.transpose(1, 3, 0, 2)).reshape(H * D, T)
    xh = xT_full.astype(np.float16)
    in_maps = []
    for c in range(N_CORES):
        r = c * HPC * D
        # wt laid out [D, HPC*D]: head h's lhsT in columns h*D:(h+1)*D
        wt = np.concatenate(
            [WT16[c * HPC + h] for h in range(HPC)], axis=1)
        in_maps.append({
            "xh": xh[r:r + HPC * D],
            "wt": np.ascontiguousarray(wt),
        })
    return in_maps


def _postprocess(results):
    """Gather per-core yT shards back into (B, H, N, D) fp32."""
    yT_full = np.concatenate([r["yT"] for r in results], axis=0)  # (H*D, T) bf16
    y = yT_full.astype(np.float32).reshape(H, D, B, N).transpose(2, 0, 3, 1)
    return np.ascontiguousarray(y)


def _execute(in_maps, trace=False, **kwargs):
    _ensure_path()
    from concourse.bass_utils import run_bass_kernel_spmd

    nc = _get_nc()
    return run_bass_kernel_spmd(nc, in_maps, core_ids=list(range(N_CORES)),
                                trace=trace, **kwargs)


def kernel(x, S_raw):
    in_maps = _prep_inputs(x, S_raw)
    res = _execute(in_maps)
    return _postprocess(res.results)
